# revision 16
# baseline (speedup 1.0000x reference)
"""AutoRegressive LSTM Trainium2 kernel (Bass/Tile), 8-way batch-parallel.

Problem: T=512, B=256, I=256, H=512, O=256 variational-dropout LSTM with
autoregressive z feedback.  Each of the 8 NeuronCores runs an independent
LSTM on its 32-sample batch shard (data parallel, no collectives).

Per-core layouts:
  r-layout for H-sized tensors: [32*G, H/G]   partition 32n+b <-> h[b, (H/G)*n + f]
  gates: 4 PSUM tiles [32*G, 512/G], filled by column-tiled matmuls
         (tile_position=(0,32n)) so G matmuls run concurrently in the PE.
  stationaries (transposed activations) produced by PE-mode transposes.
"""

import sys
import os

for _p in ("/opt/trn_rl_repo", "/root/.axon_site/_ro/trn_rl_repo"):
    if os.path.isdir(_p) and _p not in sys.path:
        sys.path.append(_p)

import numpy as np

T, B, I, H, O = 512, 256, 256, 512, 256
NCORES = 8
BL = B // NCORES          # 32 batch rows per core
NH = 4 * H                # 2048 gate columns
KC_X, KC_Z, KC_H = I // 128, O // 128, H // 128   # 2, 2, 4
KC = KC_X + KC_Z + KC_H                           # 8 contraction chunks
# chunk order in the combined weight tensor: x (0,1), z (2,3), h (4..7)
XK = list(range(0, KC_X))
ZK = list(range(KC_X, KC_X + KC_Z))
HK = list(range(KC_X + KC_Z, KC))

MODE = os.environ.get("LSTM_KERNEL_MODE", "f32r")   # "f32r" | "bf16"
UNROLL = int(os.environ.get("LSTM_KERNEL_UNROLL", "8"))

_PROGRAM_CACHE = {}


def _cfg(mode):
    import ml_dtypes
    from concourse import mybir

    if mode == "f32r":
        # float32r: fp32-width matmul format at 1 cycle/row for N>=256.
        # Producers of matmul inputs must emit float32r directly (walrus
        # verifier enforces "rounded to FP32r").
        return dict(G=2, mm_dt=mybir.dt.float32r, mm_np=np.float32,
                    bitcast=False)
    elif mode == "bf16":
        return dict(G=4, mm_dt=mybir.dt.bfloat16, mm_np=ml_dtypes.bfloat16,
                    bitcast=False)
    raise ValueError(mode)


def _hcol(k, G):
    """Column range start in the transposed-h tile for h-chunk k."""
    if G == 4:
        return 32 * k
    # G == 2: transpose half h covers chunks (0,2), second half (1,3)
    return 64 * (k % 2) + 32 * (k // 2)


# ---------------------------------------------------------------- host prep

def _to_r(a, G):
    """[BL, H] -> r-layout [32G, H/G]."""
    bl, h = a.shape
    return np.ascontiguousarray(
        a.reshape(bl, G, h // G).transpose(1, 0, 2).reshape(G * bl, h // G))


def _from_r(a, G):
    """r-layout [32G, H/G] -> [BL, H]."""
    gbl, hg = a.shape
    bl = gbl // G
    return np.ascontiguousarray(
        a.reshape(G, bl, hg).transpose(1, 0, 2).reshape(bl, G * hg))


def _hT_layout(h, G):
    """[BL, H] -> transposed-stationary tile [128, 128] matching _hcol."""
    out = np.zeros((128, 128), dtype=h.dtype)
    for k in range(KC_H):
        c = _hcol(k, G)
        out[:, c:c + 32] = h[:, 128 * k:128 * (k + 1)].T
    return out


def _zT_layout(z):
    """[BL, O] -> [128, 64] with chunk k at cols [32k:32k+32]."""
    out = np.zeros((128, 64), dtype=z.dtype)
    for k in range(KC_Z):
        out[:, 32 * k:32 * (k + 1)] = z[:, 128 * k:128 * (k + 1)].T
    return out


# ---------------------------------------------------------------- program

def _build(nsteps, mode, unroll):
    import concourse.bacc as bacc
    import concourse.bass as bass
    import concourse.tile as tile
    from concourse import mybir

    cfg = _cfg(mode)
    G = cfg["G"]
    mm_dt = cfg["mm_dt"]
    f32 = mybir.dt.float32
    RP = 32 * G          # cell partition count
    RF = H // G          # cell free size
    AF = mybir.ActivationFunctionType

    def mmc(ap):
        return ap.bitcast(mybir.dt.float32r) if cfg["bitcast"] else ap

    nc = bacc.Bacc("TRN2", target_bir_lowering=False, debug=False,
                   num_devices=NCORES)

    # ---- DRAM I/O
    xT_d = nc.dram_tensor("xT", [nsteps, 128, KC_X, BL], mm_dt,
                          kind="ExternalInput")
    wt_d = nc.dram_tensor("wt", [128, KC, NH], mm_dt, kind="ExternalInput")
    wfc_d = nc.dram_tensor("wfc", [128, KC_H, O], mm_dt, kind="ExternalInput")
    bias_d = nc.dram_tensor("bias", [1, NH], mm_dt, kind="ExternalInput")
    hT0_d = nc.dram_tensor("hT0", [128, 128], mm_dt, kind="ExternalInput")
    zT0_d = nc.dram_tensor("zT0", [128, 64], mm_dt, kind="ExternalInput")
    c0_d = nc.dram_tensor("c0r", [RP, RF], f32, kind="ExternalInput")
    cm_d = nc.dram_tensor("cmr", [RP, RF], f32, kind="ExternalInput")
    hm_d = nc.dram_tensor("hmr", [RP, RF], f32, kind="ExternalInput")
    hmT_d = nc.dram_tensor("hmT", [128, 128], f32, kind="ExternalInput")
    om_d = nc.dram_tensor("om", [BL, O], f32, kind="ExternalInput")
    ones_d = nc.dram_tensor("ones_in", [1, BL], mm_dt, kind="ExternalInput")
    id_d = nc.dram_tensor("ident", [128, 128], f32, kind="ExternalInput")

    z_out = nc.dram_tensor("z_out", [nsteps, BL, O], f32, kind="ExternalOutput")
    h_out = nc.dram_tensor("h_out", [RP, RF], f32, kind="ExternalOutput")
    c_out = nc.dram_tensor("c_out", [RP, RF], f32, kind="ExternalOutput")

    GATES = ("gf", "gi", "gg", "go")           # f, i, g, o tiles
    GFUNC = (AF.Sigmoid, AF.Sigmoid, AF.Tanh, AF.Sigmoid)
    GCOL = (512, 0, 1024, 1536)                # torch order i,f,g,o in columns

    with tile.TileContext(nc) as tc:
        import contextlib
        with contextlib.ExitStack() as stk:
            consts = stk.enter_context(tc.tile_pool(name="consts", bufs=1))
            state = stk.enter_context(tc.tile_pool(name="state", bufs=1))
            xring = stk.enter_context(tc.tile_pool(name="xring", bufs=8))
            work = stk.enter_context(tc.tile_pool(name="work", bufs=3))
            pg = stk.enter_context(tc.tile_pool(name="pg", bufs=1, space="PSUM"))
            ptr = stk.enter_context(tc.tile_pool(name="ptr", bufs=1, space="PSUM"))
            pfc = stk.enter_context(tc.tile_pool(name="pfc", bufs=1, space="PSUM"))

            # ---- constants
            wt = consts.tile([128, KC, NH], mm_dt)
            nc.sync.dma_start(out=wt[:], in_=wt_d[:])
            wfc = consts.tile([128, KC_H, O], mm_dt)
            nc.sync.dma_start(out=wfc[:], in_=wfc_d[:])
            bias = consts.tile([1, NH], mm_dt)
            nc.sync.dma_start(out=bias[:], in_=bias_d[:])
            cmr = consts.tile([RP, RF], f32)
            nc.sync.dma_start(out=cmr[:], in_=cm_d[:])
            hmr = consts.tile([RP, RF], f32)
            nc.sync.dma_start(out=hmr[:], in_=hm_d[:])
            hmT = consts.tile([128, 128], f32)
            nc.sync.dma_start(out=hmT[:], in_=hmT_d[:])
            om = consts.tile([BL, O], f32)
            nc.sync.dma_start(out=om[:], in_=om_d[:])
            ident = consts.tile([128, 128], f32)
            nc.sync.dma_start(out=ident[:], in_=id_d[:])
            ones = consts.tile([1, BL], mm_dt)
            nc.sync.dma_start(out=ones[:], in_=ones_d[:])

            # ---- persistent state
            hTm = state.tile([128, 128], mm_dt)      # masked h, transposed
            nc.sync.dma_start(out=hTm[:], in_=hT0_d[:])
            zT = state.tile([128, 64], mm_dt)        # masked z, transposed
            nc.sync.dma_start(out=zT[:], in_=zT0_d[:])
            c = state.tile([RP, RF], f32)            # cell state (masked)
            nc.sync.dma_start(out=c[:], in_=c0_d[:])
            h_raw = state.tile([RP, RF], f32)        # pre-mask h (last step)

            def step(iv):
                # -------- gate matmuls into 4 PSUM tiles [RP, RF]
                # pad each gate tile to a full PSUM bank (2 KiB/partition) so
                # their matmul zero-regions don't overlap
                g_ps = [pg.tile([RP, RF], f32, tag=t, name=t,
                                padded_shape=[128, 512]) for t in GATES]

                xt = xring.tile([128, KC_X, BL], mm_dt, tag="xt")
                nc.sync.dma_start(out=xt[:], in_=xT_d[bass.ts(iv, 1)])

                def stat(k):
                    if k in XK:
                        return xt[:, k, :]
                    if k in ZK:
                        kz = k - KC_X
                        return zT[:, 32 * kz:32 * kz + 32]
                    kh = k - KC_X - KC_Z
                    col = _hcol(kh, G)
                    return hTm[:, col:col + 32]

                # bias (start=True) via K=1 ones matmul
                for gi in range(4):
                    for n in range(G):
                        cs = GCOL[gi] + RF * n
                        # skip_group_check: CoreSim's zero-region bookkeeping
                        # mis-translates partition-offset outputs; the data
                        # path is still correct for col-tiled accumulation.
                        nc.tensor.matmul(
                            out=g_ps[gi][32 * n:32 * n + 32, :],
                            lhsT=mmc(ones[:]),
                            rhs=mmc(bias[0:1, cs:cs + RF]),
                            start=True, stop=False,
                            tile_position=(0, 32 * n),
                            skip_group_check=(n > 0),
                        )
                # contraction chunks: x first, then h, then z (z arrives last)
                order = XK + HK + ZK
                for idx, k in enumerate(order):
                    last = idx == len(order) - 1
                    st = stat(k)
                    for gi in range(4):
                        for n in range(G):
                            cs = GCOL[gi] + RF * n
                            nc.tensor.matmul(
                                out=g_ps[gi][32 * n:32 * n + 32, :],
                                lhsT=mmc(st),
                                rhs=mmc(wt[:, k, cs:cs + RF]),
                                start=False, stop=last,
                                tile_position=(0, 32 * n),
                                skip_group_check=(n > 0),
                            )

                # -------- activations
                s = []
                for gi in range(4):
                    sg = work.tile([RP, RF], f32, tag="s" + GATES[gi])
                    nc.scalar.activation(out=sg[:], in_=g_ps[gi][:],
                                         func=GFUNC[gi], bias=0.0, scale=1.0)
                    s.append(sg)
                s_f, s_i, t_g, s_o = s

                # -------- cell math (r-layout, fp32)
                tmp1 = work.tile([RP, RF], f32, tag="tmp1")
                nc.vector.tensor_mul(out=tmp1[:], in0=s_f[:], in1=c[:])
                tmp2 = work.tile([RP, RF], f32, tag="tmp2")
                nc.vector.tensor_mul(out=tmp2[:], in0=s_i[:], in1=t_g[:])
                c_raw = work.tile([RP, RF], f32, tag="c_raw")
                nc.vector.tensor_add(out=c_raw[:], in0=tmp1[:], in1=tmp2[:])
                nc.vector.tensor_mul(out=c[:], in0=c_raw[:], in1=cmr[:])
                t_c = work.tile([RP, RF], f32, tag="t_c")
                nc.scalar.activation(out=t_c[:], in_=c_raw[:],
                                     func=AF.Tanh, bias=0.0, scale=1.0)
                nc.vector.tensor_mul(out=h_raw[:], in0=s_o[:], in1=t_c[:])

                # -------- transpose h_raw -> hT psum [128, 128] (fp32)
                hT_ps = ptr.tile([128, 128], f32, tag="hT",
                                 padded_shape=[128, 512])
                if G == 4:
                    nc.tensor.matmul(out=hT_ps[:], lhsT=h_raw[:],
                                     rhs=ident[:], is_transpose=True)
                else:
                    nc.tensor.matmul(out=hT_ps[:, 0:64], lhsT=h_raw[:, 0:128],
                                     rhs=ident[0:64, 0:64], is_transpose=True,
                                     start=True, stop=False)
                    nc.tensor.matmul(out=hT_ps[:, 64:128],
                                     lhsT=h_raw[:, 128:256],
                                     rhs=ident[0:64, 0:64], is_transpose=True,
                                     start=False, stop=True)

                # evictions: unmasked (fc stationary) + masked (next gates)
                hTr = work.tile([128, 128], mm_dt, tag="hTr")
                nc.scalar.copy(out=hTr[:], in_=hT_ps[:])
                nc.vector.tensor_mul(out=hTm[:], in0=hT_ps[:], in1=hmT[:])

                # -------- fc: z_pre = h_raw @ W_fc.T   [BL, O]
                fc_ps = pfc.tile([BL, O], f32, tag="fc",
                                 padded_shape=[128, 512])
                for k in range(KC_H):
                    col = _hcol(k, G)
                    nc.tensor.matmul(
                        out=fc_ps[:],
                        lhsT=mmc(hTr[:, col:col + 32]),
                        rhs=mmc(wfc[:, k, :]),
                        start=(k == 0), stop=(k == KC_H - 1),
                        tile_position=(0, 0),
                    )
                t_fc = work.tile([BL, O], f32, tag="t_fc")
                nc.scalar.activation(out=t_fc[:], in_=fc_ps[:],
                                     func=AF.Tanh, bias=0.0, scale=1.0)
                z_m = work.tile([BL, O], f32, tag="z_m")
                nc.vector.tensor_mul(out=z_m[:], in0=t_fc[:], in1=om[:])
                nc.sync.dma_start(out=z_out[bass.ts(iv, 1)], in_=z_m[:])

                # -------- transpose z -> zT [128, 64]
                zT_ps = ptr.tile([128, 64], f32, tag="zT",
                                 padded_shape=[128, 512])
                nc.tensor.matmul(out=zT_ps[:, 0:32], lhsT=z_m[:, 0:128],
                                 rhs=ident[0:32, 0:32], is_transpose=True,
                                 start=True, stop=False)
                nc.tensor.matmul(out=zT_ps[:, 32:64], lhsT=z_m[:, 128:256],
                                 rhs=ident[0:32, 0:32], is_transpose=True,
                                 start=False, stop=True)
                nc.vector.tensor_copy(out=zT[:], in_=zT_ps[:])

            if unroll >= nsteps:
                for t_i in range(nsteps):
                    step(t_i)
            else:
                assert nsteps % unroll == 0
                tc.For_i_unrolled(0, nsteps, 1, step, max_unroll=unroll)

            # -------- final outputs
            h_m = state.tile([RP, RF], f32)
            nc.vector.tensor_mul(out=h_m[:], in0=h_raw[:], in1=hmr[:])
            nc.sync.dma_start(out=h_out[:], in_=h_m[:])
            nc.sync.dma_start(out=c_out[:], in_=c[:])

    nc.finalize()
    return nc


def _get_program(nsteps, mode, unroll):
    key = (nsteps, mode, unroll)
    if key not in _PROGRAM_CACHE:
        _PROGRAM_CACHE[key] = _build(nsteps, mode, unroll)
    return _PROGRAM_CACHE[key]


# ---------------------------------------------------------------- entry

def make_in_maps(inputs, nsteps, mode):
    """Full inputs -> per-core input dicts (host-side shard + relayout)."""
    cfg = _cfg(mode)
    G, mm_np = cfg["G"], cfg["mm_np"]

    x = np.asarray(inputs["inputs"], np.float32)[:nsteps]
    h0 = np.asarray(inputs["h0"], np.float32)
    c0 = np.asarray(inputs["c0"], np.float32)
    z0 = np.asarray(inputs["z0"], np.float32)
    om = np.asarray(inputs["out_mask"], np.float32)
    hm = np.asarray(inputs["h_mask"], np.float32)
    cm = np.asarray(inputs["c_mask"], np.float32)
    W_ih = np.asarray(inputs["W_ih"], np.float32)
    W_hh = np.asarray(inputs["W_hh"], np.float32)
    b = (np.asarray(inputs["b_ih"], np.float32)
         + np.asarray(inputs["b_hh"], np.float32))
    W_fc = np.asarray(inputs["W_fc"], np.float32)

    # combined weights, chunk order x, z, h
    W_comb = np.concatenate([W_ih[:, :I], W_ih[:, I:], W_hh], axis=1)  # [NH, 1024]
    wt = np.ascontiguousarray(
        W_comb.T.reshape(KC, 128, NH).transpose(1, 0, 2)).astype(mm_np)
    wfc = np.ascontiguousarray(
        W_fc.T.reshape(KC_H, 128, O).transpose(1, 0, 2)).astype(mm_np)
    bias = b.reshape(1, NH).astype(mm_np)
    ident = np.eye(128, dtype=np.float32)

    in_maps = []
    for j in range(NCORES):
        sl = slice(BL * j, BL * (j + 1))
        xc = x[:, sl, :]                                     # [T, BL, I]
        xT = np.ascontiguousarray(
            xc.reshape(nsteps, BL, KC_X, 128).transpose(0, 3, 2, 1)
        ).astype(mm_np)                                      # [T,128,KC_X,BL]
        in_maps.append(dict(
            xT=xT, wt=wt, wfc=wfc, bias=bias,
            hT0=_hT_layout(h0[sl], G).astype(mm_np),
            zT0=_zT_layout(z0[sl]).astype(mm_np),
            c0r=_to_r(c0[sl], G),
            cmr=_to_r(cm[sl], G),
            hmr=_to_r(hm[sl], G),
            hmT=_hT_layout(hm[sl], G).astype(np.float32),
            om=np.ascontiguousarray(om[sl]),
            ones_in=np.ones((1, BL), mm_np),
            ident=ident,
        ))
    return in_maps


def assemble_outputs(results, nsteps, mode):
    G = _cfg(mode)["G"]
    outs = np.zeros((nsteps, B, O), np.float32)
    h_f = np.zeros((B, H), np.float32)
    c_f = np.zeros((B, H), np.float32)
    for j, r in enumerate(results):
        sl = slice(BL * j, BL * (j + 1))
        outs[:, sl, :] = r["z_out"]
        h_f[sl] = _from_r(r["h_out"], G)
        c_f[sl] = _from_r(r["c_out"], G)
    return outs, (h_f, c_f)


def run(inputs, nsteps=T, mode=MODE, unroll=UNROLL, **spmd_kwargs):
    from concourse.bass_utils import run_bass_kernel_spmd

    nc = _get_program(nsteps, mode, unroll)
    in_maps = make_in_maps(inputs, nsteps, mode)
    res = run_bass_kernel_spmd(nc, in_maps, core_ids=list(range(NCORES)),
                               **spmd_kwargs)
    return res, assemble_outputs(res.results, nsteps, mode)


def kernel(**inputs):
    _res, out = run(inputs)
    return out


# revision 32
# speedup vs baseline: 1.0493x; 1.0493x over previous
"""AutoRegressive LSTM Trainium2 kernel (Bass/Tile), 8-way batch-parallel.

Problem: T=512, B=256, I=256, H=512, O=256 variational-dropout LSTM with
autoregressive z feedback.  Each of the 8 NeuronCores runs an independent
LSTM on its 32-sample batch shard (data parallel, no collectives).

Per-core layouts:
  r-layout for H-sized tensors: [32*G, H/G]   partition 32n+b <-> h[b, (H/G)*n + f]
  gates: 4 PSUM tiles [32*G, 512/G], filled by column-tiled matmuls
         (tile_position=(0,32n)) so G matmuls run concurrently in the PE.
  stationaries (transposed activations) produced by PE-mode transposes.
"""

import sys
import os

for _p in ("/opt/trn_rl_repo", "/root/.axon_site/_ro/trn_rl_repo"):
    if os.path.isdir(_p) and _p not in sys.path:
        sys.path.append(_p)

import numpy as np

T, B, I, H, O = 512, 256, 256, 512, 256
NCORES = 8
BL = B // NCORES          # 32 batch rows per core
NH = 4 * H                # 2048 gate columns
KC_X, KC_Z, KC_H = I // 128, O // 128, H // 128   # 2, 2, 4
KC = KC_X + KC_Z + KC_H                           # 8 contraction chunks
# chunk order in the combined weight tensor: x (0,1), z (2,3), h (4..7)
XK = list(range(0, KC_X))
ZK = list(range(KC_X, KC_X + KC_Z))
HK = list(range(KC_X + KC_Z, KC))

MODE = os.environ.get("LSTM_KERNEL_MODE", "bf16")   # "bf16" | "f32r"
UNROLL = int(os.environ.get("LSTM_KERNEL_UNROLL", "8"))

_PROGRAM_CACHE = {}


def _cfg(mode):
    import ml_dtypes
    from concourse import mybir

    if mode == "f32r":
        # float32r: fp32-width matmul format at 1 cycle/row for N>=256.
        # Producers of matmul inputs must emit float32r directly (walrus
        # verifier enforces "rounded to FP32r").
        return dict(G=2, mm_dt=mybir.dt.float32r, mm_np=np.float32,
                    bitcast=False)
    elif mode == "bf16":
        return dict(G=4, mm_dt=mybir.dt.bfloat16, mm_np=ml_dtypes.bfloat16,
                    bitcast=False)
    raise ValueError(mode)


def _hcol(k, G):
    """Column range start in the transposed-h tile for h-chunk k."""
    if G == 4:
        return 32 * k
    # G == 2: transpose half h covers chunks (0,2), second half (1,3)
    return 64 * (k % 2) + 32 * (k // 2)


# ---------------------------------------------------------------- host prep

def _to_r(a, G):
    """[BL, H] -> r-layout [32G, H/G]."""
    bl, h = a.shape
    return np.ascontiguousarray(
        a.reshape(bl, G, h // G).transpose(1, 0, 2).reshape(G * bl, h // G))


def _from_r(a, G):
    """r-layout [32G, H/G] -> [BL, H]."""
    gbl, hg = a.shape
    bl = gbl // G
    return np.ascontiguousarray(
        a.reshape(G, bl, hg).transpose(1, 0, 2).reshape(bl, G * hg))


def _hT_layout(h, G):
    """[BL, H] -> transposed-stationary tile [128, 128] matching _hcol."""
    out = np.zeros((128, 128), dtype=h.dtype)
    for k in range(KC_H):
        c = _hcol(k, G)
        out[:, c:c + 32] = h[:, 128 * k:128 * (k + 1)].T
    return out


def _zT_layout(z):
    """[BL, O] -> [128, 64] with chunk k at cols [32k:32k+32]."""
    out = np.zeros((128, 64), dtype=z.dtype)
    for k in range(KC_Z):
        out[:, 32 * k:32 * (k + 1)] = z[:, 128 * k:128 * (k + 1)].T
    return out


# ---------------------------------------------------------------- program

def _build(nsteps, mode, unroll):
    import concourse.bacc as bacc
    import concourse.bass as bass
    import concourse.tile as tile
    from concourse import mybir

    cfg = _cfg(mode)
    G = cfg["G"]
    assert G == 4, "v3 kernel requires the 4-group bf16 layout"
    mm_dt = cfg["mm_dt"]
    f32 = mybir.dt.float32
    RP = 32 * G          # cell partition count
    RF = H // G          # cell free size
    AF = mybir.ActivationFunctionType

    def mmc(ap):
        return ap.bitcast(mybir.dt.float32r) if cfg["bitcast"] else ap

    nc = bacc.Bacc("TRN2", target_bir_lowering=False, debug=False,
                   num_devices=NCORES)

    # ---- DRAM I/O
    xT_d = nc.dram_tensor("xT", [nsteps, 128, KC_X, BL], mm_dt,
                          kind="ExternalInput")
    # wt columns are (g,n)-swapped so that one N=512 matmul per (k, n)
    # covers all four gates of partition-group n: col' = 512n + 128g + f
    wt_d = nc.dram_tensor("wt", [128, KC, NH], mm_dt, kind="ExternalInput")
    wfc_d = nc.dram_tensor("wfc", [128, KC_H, O], mm_dt, kind="ExternalInput")
    bias_d = nc.dram_tensor("bias", [G, NH // G], mm_dt, kind="ExternalInput")
    e_d = nc.dram_tensor("emat", [G, 128], mm_dt, kind="ExternalInput")
    hT0_d = nc.dram_tensor("hT0", [128, 128], mm_dt, kind="ExternalInput")
    zT0_d = nc.dram_tensor("zT0", [128, 64], mm_dt, kind="ExternalInput")
    c0_d = nc.dram_tensor("c0r", [RP, RF], f32, kind="ExternalInput")
    cm_d = nc.dram_tensor("cmr", [RP, RF], f32, kind="ExternalInput")
    hm_d = nc.dram_tensor("hmr", [RP, RF], f32, kind="ExternalInput")
    hmT_d = nc.dram_tensor("hmT", [128, 128], f32, kind="ExternalInput")
    om_d = nc.dram_tensor("om", [BL, O], f32, kind="ExternalInput")
    id_d = nc.dram_tensor("ident", [128, 128], f32, kind="ExternalInput")

    z_out = nc.dram_tensor("z_out", [nsteps, BL, O], f32, kind="ExternalOutput")
    h_out = nc.dram_tensor("h_out", [RP, RF], f32, kind="ExternalOutput")
    c_out = nc.dram_tensor("c_out", [RP, RF], f32, kind="ExternalOutput")

    # gate order inside the reordered weight columns: i, f, g, o at
    # free-column slices [128g : 128g+128] of the combined gates tile
    GFUNC = (AF.Sigmoid, AF.Sigmoid, AF.Tanh, AF.Sigmoid)  # i, f, g, o

    with tile.TileContext(nc) as tc:
        import contextlib
        with contextlib.ExitStack() as stk:
            consts = stk.enter_context(tc.tile_pool(name="consts", bufs=1))
            state = stk.enter_context(tc.tile_pool(name="state", bufs=1))
            xring = stk.enter_context(tc.tile_pool(name="xring", bufs=8))
            work = stk.enter_context(tc.tile_pool(name="work", bufs=3))
            pg = stk.enter_context(tc.tile_pool(name="pg", bufs=2, space="PSUM"))
            ptr = stk.enter_context(tc.tile_pool(name="ptr", bufs=1, space="PSUM"))
            pfc = stk.enter_context(tc.tile_pool(name="pfc", bufs=1, space="PSUM"))

            # ---- constants
            wt = consts.tile([128, KC, NH], mm_dt)
            nc.sync.dma_start(out=wt[:], in_=wt_d[:])
            wfc = consts.tile([128, KC_H, O], mm_dt)
            nc.sync.dma_start(out=wfc[:], in_=wfc_d[:])
            bias = consts.tile([G, NH // G], mm_dt)
            nc.sync.dma_start(out=bias[:], in_=bias_d[:])
            emat = consts.tile([G, 128], mm_dt)
            nc.sync.dma_start(out=emat[:], in_=e_d[:])
            cmr = consts.tile([RP, RF], f32)
            nc.sync.dma_start(out=cmr[:], in_=cm_d[:])
            hmr = consts.tile([RP, RF], f32)
            nc.sync.dma_start(out=hmr[:], in_=hm_d[:])
            hmT = consts.tile([128, 128], f32)
            nc.sync.dma_start(out=hmT[:], in_=hmT_d[:])
            om = consts.tile([BL, O], f32)
            nc.sync.dma_start(out=om[:], in_=om_d[:])
            ident = consts.tile([128, 128], f32)
            nc.sync.dma_start(out=ident[:], in_=id_d[:])


            # ---- persistent state
            hTm = state.tile([128, 128], mm_dt)      # masked h, transposed
            nc.sync.dma_start(out=hTm[:], in_=hT0_d[:])
            zT = state.tile([128, 64], mm_dt)        # masked z, transposed
            nc.sync.dma_start(out=zT[:], in_=zT0_d[:])
            c = state.tile([RP, RF], f32)            # cell state (masked)
            nc.sync.dma_start(out=c[:], in_=c0_d[:])
            h_raw = state.tile([RP, RF], f32)        # pre-mask h (last step)

            def step(iv):
                # -------- gate matmuls into ONE PSUM tile [128, 512]
                # partition 32n+b, free col 128g+f  (gate g, hidden 128n+f)
                g_ps = pg.tile([128, 512], f32, tag="gates", name="gates")

                xt = xring.tile([128, KC_X, BL], mm_dt, tag="xt")
                nc.sync.dma_start(out=xt[:], in_=xT_d[bass.ts(iv, 1)])

                def stat(k):
                    if k in XK:
                        return xt[:, k, :]
                    if k in ZK:
                        kz = k - KC_X
                        return zT[:, 32 * kz:32 * kz + 32]
                    kh = k - KC_X - KC_Z
                    col = _hcol(kh, G)
                    return hTm[:, col:col + 32]

                # contraction chunks: x first, then h, then z (z arrives last)
                # skip_group_check: CoreSim's zero-region bookkeeping
                # mis-translates partition-offset (col-tiled) windows; the
                # data path is correct.
                order = XK + HK + ZK
                for idx, k in enumerate(order):
                    st = stat(k)
                    last = idx == len(order) - 1
                    for n in range(G):
                        nc.tensor.matmul(
                            out=g_ps[32 * n:32 * n + 32, :],
                            lhsT=mmc(st),
                            rhs=mmc(wt[:, k, 512 * n:512 * n + 512]),
                            start=(idx == 0), stop=last,
                            tile_position=(0, 32 * n),
                            # group bookkeeping windows are mis-translated for
                            # partition-offset (n>0) outputs; keep the check
                            # (and its mark/clear side effects) only on the
                            # n==0 start and stop.
                            skip_group_check=(n > 0 or (0 < idx < len(order) - 1)),
                        )
                # bias last: one full-tile K=G matmul
                # (emat[j, 32n+b] = 1 iff n == j  =>  rows 32n+b get bias[n])
                nc.tensor.matmul(out=g_ps[:], lhsT=mmc(emat[:]),
                                 rhs=mmc(bias[:]), start=False, stop=True,
                                 skip_group_check=True)

                # -------- activations (gate g at free cols [128g:128g+128])
                s = []
                for gi, gname in enumerate(("si", "sf", "tg", "so")):
                    sg = work.tile([RP, RF], f32, tag=gname)
                    nc.scalar.activation(out=sg[:],
                                         in_=g_ps[:, 128 * gi:128 * gi + 128],
                                         func=GFUNC[gi], bias=0.0, scale=1.0)
                    s.append(sg)
                s_i, s_f, t_g, s_o = s

                # -------- cell math (r-layout, fp32)
                tmp1 = work.tile([RP, RF], f32, tag="tmp1")
                nc.vector.tensor_mul(out=tmp1[:], in0=s_f[:], in1=c[:])
                tmp2 = work.tile([RP, RF], f32, tag="tmp2")
                nc.vector.tensor_mul(out=tmp2[:], in0=s_i[:], in1=t_g[:])
                c_raw = work.tile([RP, RF], f32, tag="c_raw")
                nc.vector.tensor_add(out=c_raw[:], in0=tmp1[:], in1=tmp2[:])
                nc.vector.tensor_mul(out=c[:], in0=c_raw[:], in1=cmr[:])
                t_c = work.tile([RP, RF], f32, tag="t_c")
                nc.scalar.activation(out=t_c[:], in_=c_raw[:],
                                     func=AF.Tanh, bias=0.0, scale=1.0)
                nc.vector.tensor_mul(out=h_raw[:], in0=s_o[:], in1=t_c[:])

                # -------- transpose h_raw -> hT psum [128, 128] (fp32)
                hT_ps = ptr.tile([128, 128], f32, tag="hT",
                                 padded_shape=[128, 512])
                if G == 4:
                    nc.tensor.matmul(out=hT_ps[:], lhsT=h_raw[:],
                                     rhs=ident[:], is_transpose=True)
                else:
                    nc.tensor.matmul(out=hT_ps[:, 0:64], lhsT=h_raw[:, 0:128],
                                     rhs=ident[0:64, 0:64], is_transpose=True,
                                     start=True, stop=False)
                    nc.tensor.matmul(out=hT_ps[:, 64:128],
                                     lhsT=h_raw[:, 128:256],
                                     rhs=ident[0:64, 0:64], is_transpose=True,
                                     start=False, stop=True)

                # evictions: unmasked (fc stationary) + masked (next gates)
                hTr = work.tile([128, 128], mm_dt, tag="hTr")
                nc.scalar.copy(out=hTr[:], in_=hT_ps[:])
                nc.vector.tensor_mul(out=hTm[:], in0=hT_ps[:], in1=hmT[:])

                # -------- fc: z_pre = h_raw @ W_fc.T   [BL, O]
                fc_ps = pfc.tile([BL, O], f32, tag="fc",
                                 padded_shape=[128, 512])
                for k in range(KC_H):
                    col = _hcol(k, G)
                    nc.tensor.matmul(
                        out=fc_ps[:],
                        lhsT=mmc(hTr[:, col:col + 32]),
                        rhs=mmc(wfc[:, k, :]),
                        start=(k == 0), stop=(k == KC_H - 1),
                        tile_position=(0, 0),
                    )
                t_fc = work.tile([BL, O], f32, tag="t_fc")
                nc.scalar.activation(out=t_fc[:], in_=fc_ps[:],
                                     func=AF.Tanh, bias=0.0, scale=1.0)
                z_m = work.tile([BL, O], f32, tag="z_m")
                nc.vector.tensor_mul(out=z_m[:], in0=t_fc[:], in1=om[:])
                nc.sync.dma_start(out=z_out[bass.ts(iv, 1)], in_=z_m[:])

                # -------- transpose z -> zT [128, 64]
                zT_ps = ptr.tile([128, 64], f32, tag="zT",
                                 padded_shape=[128, 512])
                nc.tensor.matmul(out=zT_ps[:, 0:32], lhsT=z_m[:, 0:128],
                                 rhs=ident[0:32, 0:32], is_transpose=True,
                                 start=True, stop=False)
                nc.tensor.matmul(out=zT_ps[:, 32:64], lhsT=z_m[:, 128:256],
                                 rhs=ident[0:32, 0:32], is_transpose=True,
                                 start=False, stop=True)
                nc.vector.tensor_copy(out=zT[:], in_=zT_ps[:])

            if unroll >= nsteps:
                for t_i in range(nsteps):
                    step(t_i)
            else:
                assert nsteps % unroll == 0
                tc.For_i_unrolled(0, nsteps, 1, step, max_unroll=unroll)

            # -------- final outputs
            h_m = state.tile([RP, RF], f32)
            nc.vector.tensor_mul(out=h_m[:], in0=h_raw[:], in1=hmr[:])
            nc.sync.dma_start(out=h_out[:], in_=h_m[:])
            nc.sync.dma_start(out=c_out[:], in_=c[:])

    nc.finalize()
    return nc


def _get_program(nsteps, mode, unroll):
    key = (nsteps, mode, unroll)
    if key not in _PROGRAM_CACHE:
        _PROGRAM_CACHE[key] = _build(nsteps, mode, unroll)
    return _PROGRAM_CACHE[key]


# ---------------------------------------------------------------- entry

def make_in_maps(inputs, nsteps, mode):
    """Full inputs -> per-core input dicts (host-side shard + relayout)."""
    cfg = _cfg(mode)
    G, mm_np = cfg["G"], cfg["mm_np"]

    x = np.asarray(inputs["inputs"], np.float32)[:nsteps]
    h0 = np.asarray(inputs["h0"], np.float32)
    c0 = np.asarray(inputs["c0"], np.float32)
    z0 = np.asarray(inputs["z0"], np.float32)
    om = np.asarray(inputs["out_mask"], np.float32)
    hm = np.asarray(inputs["h_mask"], np.float32)
    cm = np.asarray(inputs["c_mask"], np.float32)
    W_ih = np.asarray(inputs["W_ih"], np.float32)
    W_hh = np.asarray(inputs["W_hh"], np.float32)
    b = (np.asarray(inputs["b_ih"], np.float32)
         + np.asarray(inputs["b_hh"], np.float32))
    W_fc = np.asarray(inputs["W_fc"], np.float32)

    # combined weights, chunk order x, z, h
    W_comb = np.concatenate([W_ih[:, :I], W_ih[:, I:], W_hh], axis=1)  # [NH, 1024]
    # (g, n) swap of the gate-column axis: new col 512n+128g+f = old 512g+128n+f
    W_comb = np.ascontiguousarray(
        W_comb.reshape(4, G, NH // (4 * G), I + O + H)
        .transpose(1, 0, 2, 3).reshape(NH, I + O + H))
    b_r = np.ascontiguousarray(
        b.reshape(4, G, NH // (4 * G)).transpose(1, 0, 2).reshape(G, NH // G))
    wt = np.ascontiguousarray(
        W_comb.T.reshape(KC, 128, NH).transpose(1, 0, 2)).astype(mm_np)
    wfc = np.ascontiguousarray(
        W_fc.T.reshape(KC_H, 128, O).transpose(1, 0, 2)).astype(mm_np)
    bias = b_r.astype(mm_np)
    emat = np.zeros((G, 128), np.float32)
    for j in range(G):
        emat[j, 32 * j:32 * j + 32] = 1.0
    ident = np.eye(128, dtype=np.float32)

    in_maps = []
    for j in range(NCORES):
        sl = slice(BL * j, BL * (j + 1))
        xc = x[:, sl, :]                                     # [T, BL, I]
        xT = np.ascontiguousarray(
            xc.reshape(nsteps, BL, KC_X, 128).transpose(0, 3, 2, 1)
        ).astype(mm_np)                                      # [T,128,KC_X,BL]
        in_maps.append(dict(
            xT=xT, wt=wt, wfc=wfc, bias=bias, emat=emat.astype(mm_np),
            hT0=_hT_layout(h0[sl], G).astype(mm_np),
            zT0=_zT_layout(z0[sl]).astype(mm_np),
            c0r=_to_r(c0[sl], G),
            cmr=_to_r(cm[sl], G),
            hmr=_to_r(hm[sl], G),
            hmT=_hT_layout(hm[sl], G).astype(np.float32),
            om=np.ascontiguousarray(om[sl]),
            ident=ident,
        ))
    return in_maps


def assemble_outputs(results, nsteps, mode):
    G = _cfg(mode)["G"]
    outs = np.zeros((nsteps, B, O), np.float32)
    h_f = np.zeros((B, H), np.float32)
    c_f = np.zeros((B, H), np.float32)
    for j, r in enumerate(results):
        sl = slice(BL * j, BL * (j + 1))
        outs[:, sl, :] = r["z_out"]
        h_f[sl] = _from_r(r["h_out"], G)
        c_f[sl] = _from_r(r["c_out"], G)
    return outs, (h_f, c_f)


def run(inputs, nsteps=T, mode=MODE, unroll=UNROLL, **spmd_kwargs):
    from concourse.bass_utils import run_bass_kernel_spmd

    nc = _get_program(nsteps, mode, unroll)
    in_maps = make_in_maps(inputs, nsteps, mode)
    res = run_bass_kernel_spmd(nc, in_maps, core_ids=list(range(NCORES)),
                               **spmd_kwargs)
    return res, assemble_outputs(res.results, nsteps, mode)


def kernel(**inputs):
    _res, out = run(inputs)
    return out


# revision 40
# speedup vs baseline: 1.0617x; 1.0118x over previous
"""AutoRegressive LSTM Trainium2 kernel (Bass/Tile), 8-way batch-parallel.

Problem: T=512, B=256, I=256, H=512, O=256 variational-dropout LSTM with
autoregressive z feedback.  Each of the 8 NeuronCores runs an independent
LSTM on its 32-sample batch shard (data parallel, no collectives).

Per-core layouts:
  r-layout for H-sized tensors: [32*G, H/G]   partition 32n+b <-> h[b, (H/G)*n + f]
  gates: 4 PSUM tiles [32*G, 512/G], filled by column-tiled matmuls
         (tile_position=(0,32n)) so G matmuls run concurrently in the PE.
  stationaries (transposed activations) produced by PE-mode transposes.
"""

import sys
import os

for _p in ("/opt/trn_rl_repo", "/root/.axon_site/_ro/trn_rl_repo"):
    if os.path.isdir(_p) and _p not in sys.path:
        sys.path.append(_p)

import numpy as np

T, B, I, H, O = 512, 256, 256, 512, 256
NCORES = 8
BL = B // NCORES          # 32 batch rows per core
NH = 4 * H                # 2048 gate columns
KC_X, KC_Z, KC_H = I // 128, O // 128, H // 128   # 2, 2, 4
KC = KC_X + KC_Z + KC_H                           # 8 contraction chunks
# chunk order in the combined weight tensor: x (0,1), z (2,3), h (4..7)
XK = list(range(0, KC_X))
ZK = list(range(KC_X, KC_X + KC_Z))
HK = list(range(KC_X + KC_Z, KC))

MODE = os.environ.get("LSTM_KERNEL_MODE", "bf16")   # "bf16" | "f32r"
UNROLL = int(os.environ.get("LSTM_KERNEL_UNROLL", "16"))
# scheduler-priority hoist (~2 steps' worth of instructions) for the
# x-chunk matmuls, letting the PE run them during the cell-chain stall
PRIO_HOIST = 140

_PROGRAM_CACHE = {}


def _cfg(mode):
    import ml_dtypes
    from concourse import mybir

    if mode == "f32r":
        # float32r: fp32-width matmul format at 1 cycle/row for N>=256.
        # Producers of matmul inputs must emit float32r directly (walrus
        # verifier enforces "rounded to FP32r").
        return dict(G=2, mm_dt=mybir.dt.float32r, mm_np=np.float32,
                    bitcast=False)
    elif mode == "bf16":
        return dict(G=4, mm_dt=mybir.dt.bfloat16, mm_np=ml_dtypes.bfloat16,
                    bitcast=False)
    raise ValueError(mode)


def _hcol(k, G):
    """Column range start in the transposed-h tile for h-chunk k."""
    if G == 4:
        return 32 * k
    # G == 2: transpose half h covers chunks (0,2), second half (1,3)
    return 64 * (k % 2) + 32 * (k // 2)


# ---------------------------------------------------------------- host prep

def _to_r(a, G):
    """[BL, H] -> r-layout [32G, H/G]."""
    bl, h = a.shape
    return np.ascontiguousarray(
        a.reshape(bl, G, h // G).transpose(1, 0, 2).reshape(G * bl, h // G))


def _from_r(a, G):
    """r-layout [32G, H/G] -> [BL, H]."""
    gbl, hg = a.shape
    bl = gbl // G
    return np.ascontiguousarray(
        a.reshape(G, bl, hg).transpose(1, 0, 2).reshape(bl, G * hg))


def _hT_layout(h, G):
    """[BL, H] -> transposed-stationary tile [128, 128] matching _hcol."""
    out = np.zeros((128, 128), dtype=h.dtype)
    for k in range(KC_H):
        c = _hcol(k, G)
        out[:, c:c + 32] = h[:, 128 * k:128 * (k + 1)].T
    return out


def _zT_layout(z):
    """[BL, O] -> [128, 64] with chunk k at cols [32k:32k+32]."""
    out = np.zeros((128, 64), dtype=z.dtype)
    for k in range(KC_Z):
        out[:, 32 * k:32 * (k + 1)] = z[:, 128 * k:128 * (k + 1)].T
    return out


# ---------------------------------------------------------------- program

def _build(nsteps, mode, unroll):
    import concourse.bacc as bacc
    import concourse.bass as bass
    import concourse.tile as tile
    from concourse import mybir

    cfg = _cfg(mode)
    G = cfg["G"]
    assert G == 4, "v3 kernel requires the 4-group bf16 layout"
    mm_dt = cfg["mm_dt"]
    f32 = mybir.dt.float32
    RP = 32 * G          # cell partition count
    RF = H // G          # cell free size
    AF = mybir.ActivationFunctionType

    def mmc(ap):
        return ap.bitcast(mybir.dt.float32r) if cfg["bitcast"] else ap

    nc = bacc.Bacc("TRN2", target_bir_lowering=False, debug=False,
                   num_devices=NCORES)

    # ---- DRAM I/O
    xT_d = nc.dram_tensor("xT", [nsteps, 128, KC_X, BL], mm_dt,
                          kind="ExternalInput")
    # wt columns are (g,n)-swapped so that one N=512 matmul per (k, n)
    # covers all four gates of partition-group n: col' = 512n + 128g + f
    wt_d = nc.dram_tensor("wt", [128, KC, NH], mm_dt, kind="ExternalInput")
    wfc_d = nc.dram_tensor("wfc", [128, KC_H, O], mm_dt, kind="ExternalInput")
    bias_d = nc.dram_tensor("bias", [G, NH // G], mm_dt, kind="ExternalInput")
    e_d = nc.dram_tensor("emat", [G, 128], mm_dt, kind="ExternalInput")
    hT0_d = nc.dram_tensor("hT0", [128, 128], mm_dt, kind="ExternalInput")
    zT0_d = nc.dram_tensor("zT0", [128, 64], mm_dt, kind="ExternalInput")
    c0_d = nc.dram_tensor("c0r", [RP, RF], f32, kind="ExternalInput")
    cm_d = nc.dram_tensor("cmr", [RP, RF], f32, kind="ExternalInput")
    hm_d = nc.dram_tensor("hmr", [RP, RF], f32, kind="ExternalInput")
    hmT_d = nc.dram_tensor("hmT", [128, 128], f32, kind="ExternalInput")
    om_d = nc.dram_tensor("om", [BL, O], f32, kind="ExternalInput")
    id_d = nc.dram_tensor("ident", [128, 128], f32, kind="ExternalInput")

    z_out = nc.dram_tensor("z_out", [nsteps, BL, O], f32, kind="ExternalOutput")
    h_out = nc.dram_tensor("h_out", [RP, RF], f32, kind="ExternalOutput")
    c_out = nc.dram_tensor("c_out", [RP, RF], f32, kind="ExternalOutput")

    # gate order inside the reordered weight columns is (i, f, o, g) so one
    # sigmoid covers free cols [0:384] and one tanh covers [384:512]

    with tile.TileContext(nc) as tc:
        import contextlib
        with contextlib.ExitStack() as stk:
            consts = stk.enter_context(tc.tile_pool(name="consts", bufs=1))
            state = stk.enter_context(tc.tile_pool(name="state", bufs=1))
            xring = stk.enter_context(tc.tile_pool(name="xring", bufs=8))
            work = stk.enter_context(tc.tile_pool(name="work", bufs=3))
            pg = stk.enter_context(tc.tile_pool(name="pg", bufs=2, space="PSUM"))
            ptr = stk.enter_context(tc.tile_pool(name="ptr", bufs=1, space="PSUM"))
            pfc = stk.enter_context(tc.tile_pool(name="pfc", bufs=1, space="PSUM"))

            # ---- constants
            wt = consts.tile([128, KC, NH], mm_dt)
            nc.sync.dma_start(out=wt[:], in_=wt_d[:])
            wfc = consts.tile([128, KC_H, O], mm_dt)
            nc.sync.dma_start(out=wfc[:], in_=wfc_d[:])
            bias = consts.tile([G, NH // G], mm_dt)
            nc.sync.dma_start(out=bias[:], in_=bias_d[:])
            emat = consts.tile([G, 128], mm_dt)
            nc.sync.dma_start(out=emat[:], in_=e_d[:])
            cmr = consts.tile([RP, RF], f32)
            nc.sync.dma_start(out=cmr[:], in_=cm_d[:])
            hmr = consts.tile([RP, RF], f32)
            nc.sync.dma_start(out=hmr[:], in_=hm_d[:])
            hmT = consts.tile([128, 128], f32)
            nc.sync.dma_start(out=hmT[:], in_=hmT_d[:])
            om = consts.tile([BL, O], f32)
            nc.sync.dma_start(out=om[:], in_=om_d[:])
            ident = consts.tile([128, 128], f32)
            nc.sync.dma_start(out=ident[:], in_=id_d[:])


            # ---- persistent state
            hTm = state.tile([128, 128], mm_dt)      # masked h, transposed
            nc.sync.dma_start(out=hTm[:], in_=hT0_d[:])
            zT = state.tile([128, 64], mm_dt)        # masked z, transposed
            nc.sync.dma_start(out=zT[:], in_=zT0_d[:])
            c = state.tile([RP, RF], f32)            # cell state (masked)
            nc.sync.dma_start(out=c[:], in_=c0_d[:])
            h_raw = state.tile([RP, RF], f32)        # pre-mask h (last step)

            def step(iv):
                # -------- gate matmuls into ONE PSUM tile [128, 512]
                # partition 32n+b, free col 128g+f  (gate g, hidden 128n+f)
                g_ps = pg.tile([128, 512], f32, tag="gates", name="gates")

                xt = xring.tile([128, KC_X, BL], mm_dt, tag="xt")
                with tc.high_priority(offset=PRIO_HOIST):
                    nc.sync.dma_start(out=xt[:], in_=xT_d[bass.ts(iv, 1)])

                def stat(k):
                    if k in XK:
                        return xt[:, k, :]
                    if k in ZK:
                        kz = k - KC_X
                        return zT[:, 32 * kz:32 * kz + 32]
                    kh = k - KC_X - KC_Z
                    col = _hcol(kh, G)
                    return hTm[:, col:col + 32]

                # contraction chunks: x first, then h, then z (z arrives last)
                # skip_group_check: CoreSim's zero-region bookkeeping
                # mis-translates partition-offset (col-tiled) windows; the
                # data path is correct.
                order = XK + HK + ZK
                for idx, k in enumerate(order):
                    st = stat(k)
                    last = idx == len(order) - 1
                    import contextlib as _cl
                    # hoist the x chunks (no recurrent deps) so the PE can run
                    # them while the previous step's cell chain is in flight
                    hp = (tc.high_priority(offset=PRIO_HOIST) if k in XK
                          else _cl.nullcontext())
                    with hp:
                        for n in range(G):
                            nc.tensor.matmul(
                                out=g_ps[32 * n:32 * n + 32, :],
                                lhsT=mmc(st),
                                rhs=mmc(wt[:, k, 512 * n:512 * n + 512]),
                                start=(idx == 0), stop=last,
                                tile_position=(0, 32 * n),
                                # group bookkeeping windows are mis-translated
                                # for partition-offset (n>0) outputs; keep the
                                # check (and its mark/clear side effects) only
                                # on the n==0 start and stop.
                                skip_group_check=(
                                    n > 0 or (0 < idx < len(order) - 1)),
                            )
                # bias last: one full-tile K=G matmul
                # (emat[j, 32n+b] = 1 iff n == j  =>  rows 32n+b get bias[n])
                nc.tensor.matmul(out=g_ps[:], lhsT=mmc(emat[:]),
                                 rhs=mmc(bias[:]), start=False, stop=True,
                                 skip_group_check=True)

                # -------- activations: gates at free cols (i,f,o,g) order;
                # one sigmoid over [0:384], one tanh over [384:512]
                s_ifo = work.tile([RP, 384], f32, tag="sifo")
                nc.scalar.activation(out=s_ifo[:], in_=g_ps[:, 0:384],
                                     func=AF.Sigmoid, bias=0.0, scale=1.0)
                t_g = work.tile([RP, RF], f32, tag="tg")
                nc.scalar.activation(out=t_g[:], in_=g_ps[:, 384:512],
                                     func=AF.Tanh, bias=0.0, scale=1.0)
                s_i = s_ifo[:, 0:128]
                s_f = s_ifo[:, 128:256]
                s_o = s_ifo[:, 256:384]

                # -------- cell math (r-layout, fp32)
                tmp1 = work.tile([RP, RF], f32, tag="tmp1")
                nc.gpsimd.tensor_mul(out=tmp1[:], in0=s_f[:], in1=c[:])
                tmp2 = work.tile([RP, RF], f32, tag="tmp2")
                nc.vector.tensor_mul(out=tmp2[:], in0=s_i[:], in1=t_g[:])
                c_raw = work.tile([RP, RF], f32, tag="c_raw")
                nc.vector.tensor_add(out=c_raw[:], in0=tmp1[:], in1=tmp2[:])
                nc.gpsimd.tensor_mul(out=c[:], in0=c_raw[:], in1=cmr[:])
                t_c = work.tile([RP, RF], f32, tag="t_c")
                nc.scalar.activation(out=t_c[:], in_=c_raw[:],
                                     func=AF.Tanh, bias=0.0, scale=1.0)
                nc.vector.tensor_mul(out=h_raw[:], in0=s_o[:], in1=t_c[:])

                # -------- transpose h_raw -> hT psum [128, 128] (fp32)
                hT_ps = ptr.tile([128, 128], f32, tag="hT",
                                 padded_shape=[128, 512])
                if G == 4:
                    nc.tensor.matmul(out=hT_ps[:], lhsT=h_raw[:],
                                     rhs=ident[:], is_transpose=True)
                else:
                    nc.tensor.matmul(out=hT_ps[:, 0:64], lhsT=h_raw[:, 0:128],
                                     rhs=ident[0:64, 0:64], is_transpose=True,
                                     start=True, stop=False)
                    nc.tensor.matmul(out=hT_ps[:, 64:128],
                                     lhsT=h_raw[:, 128:256],
                                     rhs=ident[0:64, 0:64], is_transpose=True,
                                     start=False, stop=True)

                # evictions: unmasked (fc stationary) + masked (next gates)
                hTr = work.tile([128, 128], mm_dt, tag="hTr")
                nc.scalar.copy(out=hTr[:], in_=hT_ps[:])
                nc.vector.tensor_mul(out=hTm[:], in0=hT_ps[:], in1=hmT[:])

                # -------- fc: z_pre = h_raw @ W_fc.T   [BL, O]
                fc_ps = pfc.tile([BL, O], f32, tag="fc",
                                 padded_shape=[128, 512])
                for k in range(KC_H):
                    col = _hcol(k, G)
                    nc.tensor.matmul(
                        out=fc_ps[:],
                        lhsT=mmc(hTr[:, col:col + 32]),
                        rhs=mmc(wfc[:, k, :]),
                        start=(k == 0), stop=(k == KC_H - 1),
                        tile_position=(0, 0),
                    )
                t_fc = work.tile([BL, O], f32, tag="t_fc")
                nc.scalar.activation(out=t_fc[:], in_=fc_ps[:],
                                     func=AF.Tanh, bias=0.0, scale=1.0)
                z_m = work.tile([BL, O], f32, tag="z_m")
                nc.vector.tensor_mul(out=z_m[:], in0=t_fc[:], in1=om[:])
                nc.sync.dma_start(out=z_out[bass.ts(iv, 1)], in_=z_m[:])

                # -------- transpose z -> zT [128, 64]
                zT_ps = ptr.tile([128, 64], f32, tag="zT",
                                 padded_shape=[128, 512])
                nc.tensor.matmul(out=zT_ps[:, 0:32], lhsT=z_m[:, 0:128],
                                 rhs=ident[0:32, 0:32], is_transpose=True,
                                 start=True, stop=False)
                nc.tensor.matmul(out=zT_ps[:, 32:64], lhsT=z_m[:, 128:256],
                                 rhs=ident[0:32, 0:32], is_transpose=True,
                                 start=False, stop=True)
                nc.vector.tensor_copy(out=zT[:], in_=zT_ps[:])

            if unroll >= nsteps:
                for t_i in range(nsteps):
                    step(t_i)
            else:
                assert nsteps % unroll == 0
                tc.For_i_unrolled(0, nsteps, 1, step, max_unroll=unroll)

            # -------- final outputs
            h_m = state.tile([RP, RF], f32)
            nc.vector.tensor_mul(out=h_m[:], in0=h_raw[:], in1=hmr[:])
            nc.sync.dma_start(out=h_out[:], in_=h_m[:])
            nc.sync.dma_start(out=c_out[:], in_=c[:])

    nc.finalize()
    return nc


def _get_program(nsteps, mode, unroll):
    key = (nsteps, mode, unroll)
    if key not in _PROGRAM_CACHE:
        _PROGRAM_CACHE[key] = _build(nsteps, mode, unroll)
    return _PROGRAM_CACHE[key]


# ---------------------------------------------------------------- entry

def make_in_maps(inputs, nsteps, mode):
    """Full inputs -> per-core input dicts (host-side shard + relayout)."""
    cfg = _cfg(mode)
    G, mm_np = cfg["G"], cfg["mm_np"]

    x = np.asarray(inputs["inputs"], np.float32)[:nsteps]
    h0 = np.asarray(inputs["h0"], np.float32)
    c0 = np.asarray(inputs["c0"], np.float32)
    z0 = np.asarray(inputs["z0"], np.float32)
    om = np.asarray(inputs["out_mask"], np.float32)
    hm = np.asarray(inputs["h_mask"], np.float32)
    cm = np.asarray(inputs["c_mask"], np.float32)
    W_ih = np.asarray(inputs["W_ih"], np.float32)
    W_hh = np.asarray(inputs["W_hh"], np.float32)
    b = (np.asarray(inputs["b_ih"], np.float32)
         + np.asarray(inputs["b_hh"], np.float32))
    W_fc = np.asarray(inputs["W_fc"], np.float32)

    # combined weights, chunk order x, z, h
    W_comb = np.concatenate([W_ih[:, :I], W_ih[:, I:], W_hh], axis=1)  # [NH, 1024]
    # gate reorder (i,f,g,o)->(i,f,o,g) so one sigmoid spans 3 gates, then
    # (g, n) swap of the gate-column axis: new col 512n+128g'+f
    GP = [0, 1, 3, 2]
    W_comb = np.ascontiguousarray(
        W_comb.reshape(4, G, NH // (4 * G), I + O + H)[GP]
        .transpose(1, 0, 2, 3).reshape(NH, I + O + H))
    b_r = np.ascontiguousarray(
        b.reshape(4, G, NH // (4 * G))[GP].transpose(1, 0, 2).reshape(G, NH // G))
    wt = np.ascontiguousarray(
        W_comb.T.reshape(KC, 128, NH).transpose(1, 0, 2)).astype(mm_np)
    wfc = np.ascontiguousarray(
        W_fc.T.reshape(KC_H, 128, O).transpose(1, 0, 2)).astype(mm_np)
    bias = b_r.astype(mm_np)
    emat = np.zeros((G, 128), np.float32)
    for j in range(G):
        emat[j, 32 * j:32 * j + 32] = 1.0
    ident = np.eye(128, dtype=np.float32)

    in_maps = []
    for j in range(NCORES):
        sl = slice(BL * j, BL * (j + 1))
        xc = x[:, sl, :]                                     # [T, BL, I]
        xT = np.ascontiguousarray(
            xc.reshape(nsteps, BL, KC_X, 128).transpose(0, 3, 2, 1)
        ).astype(mm_np)                                      # [T,128,KC_X,BL]
        in_maps.append(dict(
            xT=xT, wt=wt, wfc=wfc, bias=bias, emat=emat.astype(mm_np),
            hT0=_hT_layout(h0[sl], G).astype(mm_np),
            zT0=_zT_layout(z0[sl]).astype(mm_np),
            c0r=_to_r(c0[sl], G),
            cmr=_to_r(cm[sl], G),
            hmr=_to_r(hm[sl], G),
            hmT=_hT_layout(hm[sl], G).astype(np.float32),
            om=np.ascontiguousarray(om[sl]),
            ident=ident,
        ))
    return in_maps


def assemble_outputs(results, nsteps, mode):
    G = _cfg(mode)["G"]
    outs = np.zeros((nsteps, B, O), np.float32)
    h_f = np.zeros((B, H), np.float32)
    c_f = np.zeros((B, H), np.float32)
    for j, r in enumerate(results):
        sl = slice(BL * j, BL * (j + 1))
        outs[:, sl, :] = r["z_out"]
        h_f[sl] = _from_r(r["h_out"], G)
        c_f[sl] = _from_r(r["c_out"], G)
    return outs, (h_f, c_f)


def run(inputs, nsteps=T, mode=MODE, unroll=UNROLL, **spmd_kwargs):
    from concourse.bass_utils import run_bass_kernel_spmd

    nc = _get_program(nsteps, mode, unroll)
    in_maps = make_in_maps(inputs, nsteps, mode)
    res = run_bass_kernel_spmd(nc, in_maps, core_ids=list(range(NCORES)),
                               **spmd_kwargs)
    return res, assemble_outputs(res.results, nsteps, mode)


def kernel(**inputs):
    _res, out = run(inputs)
    return out


# revision 41
# speedup vs baseline: 1.1113x; 1.0467x over previous
"""AutoRegressive LSTM Trainium2 kernel (Bass/Tile), 8-way batch-parallel.

Problem: T=512, B=256, I=256, H=512, O=256 variational-dropout LSTM with
autoregressive z feedback.  Each of the 8 NeuronCores runs an independent
LSTM on its 32-sample batch shard (data parallel, no collectives).

Per-core layouts:
  r-layout for H-sized tensors: [32*G, H/G]   partition 32n+b <-> h[b, (H/G)*n + f]
  gates: 4 PSUM tiles [32*G, 512/G], filled by column-tiled matmuls
         (tile_position=(0,32n)) so G matmuls run concurrently in the PE.
  stationaries (transposed activations) produced by PE-mode transposes.
"""

import sys
import os

for _p in ("/opt/trn_rl_repo", "/root/.axon_site/_ro/trn_rl_repo"):
    if os.path.isdir(_p) and _p not in sys.path:
        sys.path.append(_p)

import numpy as np

T, B, I, H, O = 512, 256, 256, 512, 256
NCORES = 8
BL = B // NCORES          # 32 batch rows per core
NH = 4 * H                # 2048 gate columns
KC_X, KC_Z, KC_H = I // 128, O // 128, H // 128   # 2, 2, 4
KC = KC_X + KC_Z + KC_H                           # 8 contraction chunks
# chunk order in the combined weight tensor: x (0,1), z (2,3), h (4..7)
XK = list(range(0, KC_X))
ZK = list(range(KC_X, KC_X + KC_Z))
HK = list(range(KC_X + KC_Z, KC))

MODE = os.environ.get("LSTM_KERNEL_MODE", "bf16")   # "bf16" | "f32r"
UNROLL = int(os.environ.get("LSTM_KERNEL_UNROLL", "16"))
# scheduler-priority hoist (~2 steps' worth of instructions) for the
# x-chunk matmuls, letting the PE run them during the cell-chain stall
PRIO_HOIST = 140

_PROGRAM_CACHE = {}


def _cfg(mode):
    import ml_dtypes
    from concourse import mybir

    if mode == "f32r":
        # float32r: fp32-width matmul format at 1 cycle/row for N>=256.
        # Producers of matmul inputs must emit float32r directly (walrus
        # verifier enforces "rounded to FP32r").
        return dict(G=2, mm_dt=mybir.dt.float32r, mm_np=np.float32,
                    bitcast=False)
    elif mode == "bf16":
        return dict(G=4, mm_dt=mybir.dt.bfloat16, mm_np=ml_dtypes.bfloat16,
                    bitcast=False)
    raise ValueError(mode)


def _hcol(k, G):
    """Column range start in the transposed-h tile for h-chunk k."""
    if G == 4:
        return 32 * k
    # G == 2: transpose half h covers chunks (0,2), second half (1,3)
    return 64 * (k % 2) + 32 * (k // 2)


# ---------------------------------------------------------------- host prep

def _to_r(a, G):
    """[BL, H] -> r-layout [32G, H/G]."""
    bl, h = a.shape
    return np.ascontiguousarray(
        a.reshape(bl, G, h // G).transpose(1, 0, 2).reshape(G * bl, h // G))


def _from_r(a, G):
    """r-layout [32G, H/G] -> [BL, H]."""
    gbl, hg = a.shape
    bl = gbl // G
    return np.ascontiguousarray(
        a.reshape(G, bl, hg).transpose(1, 0, 2).reshape(bl, G * hg))


def _hT_layout(h, G):
    """[BL, H] -> transposed-stationary tile [128, 128] matching _hcol."""
    out = np.zeros((128, 128), dtype=h.dtype)
    for k in range(KC_H):
        c = _hcol(k, G)
        out[:, c:c + 32] = h[:, 128 * k:128 * (k + 1)].T
    return out


def _zT_layout(z):
    """[BL, O] -> [128, 64] with chunk k at cols [32k:32k+32]."""
    out = np.zeros((128, 64), dtype=z.dtype)
    for k in range(KC_Z):
        out[:, 32 * k:32 * (k + 1)] = z[:, 128 * k:128 * (k + 1)].T
    return out


# ---------------------------------------------------------------- program

def _build(nsteps, mode, unroll):
    import concourse.bacc as bacc
    import concourse.bass as bass
    import concourse.tile as tile
    from concourse import mybir

    cfg = _cfg(mode)
    G = cfg["G"]
    assert G == 4, "v3 kernel requires the 4-group bf16 layout"
    mm_dt = cfg["mm_dt"]
    f32 = mybir.dt.float32
    RP = 32 * G          # cell partition count
    RF = H // G          # cell free size
    AF = mybir.ActivationFunctionType

    def mmc(ap):
        return ap.bitcast(mybir.dt.float32r) if cfg["bitcast"] else ap

    nc = bacc.Bacc("TRN2", target_bir_lowering=False, debug=False,
                   num_devices=NCORES)

    # ---- DRAM I/O
    xT_d = nc.dram_tensor("xT", [nsteps, 128, KC_X, BL], mm_dt,
                          kind="ExternalInput")
    # wt columns are (g,n)-swapped so that one N=512 matmul per (k, n)
    # covers all four gates of partition-group n: col' = 512n + 128g + f
    wt_d = nc.dram_tensor("wt", [128, KC, NH], mm_dt, kind="ExternalInput")
    wfc_d = nc.dram_tensor("wfc", [128, KC_H, O], mm_dt, kind="ExternalInput")
    bias_d = nc.dram_tensor("bias", [G, NH // G], mm_dt, kind="ExternalInput")
    e_d = nc.dram_tensor("emat", [G, 128], mm_dt, kind="ExternalInput")
    hT0_d = nc.dram_tensor("hT0", [128, 128], mm_dt, kind="ExternalInput")
    zT0_d = nc.dram_tensor("zT0", [128, 64], mm_dt, kind="ExternalInput")
    c0_d = nc.dram_tensor("c0r", [RP, RF], f32, kind="ExternalInput")
    cm_d = nc.dram_tensor("cmr", [RP, RF], f32, kind="ExternalInput")
    hm_d = nc.dram_tensor("hmr", [RP, RF], f32, kind="ExternalInput")
    hmT_d = nc.dram_tensor("hmT", [128, 128], f32, kind="ExternalInput")
    om_d = nc.dram_tensor("om", [BL, O], f32, kind="ExternalInput")
    id_d = nc.dram_tensor("ident", [128, 128], f32, kind="ExternalInput")

    z_out = nc.dram_tensor("z_out", [nsteps, BL, O], f32, kind="ExternalOutput")
    h_out = nc.dram_tensor("h_out", [RP, RF], f32, kind="ExternalOutput")
    c_out = nc.dram_tensor("c_out", [RP, RF], f32, kind="ExternalOutput")

    # gate order inside the reordered weight columns is (i, f, o, g) so one
    # sigmoid covers free cols [0:384] and one tanh covers [384:512]

    with tile.TileContext(nc) as tc:
        import contextlib
        with contextlib.ExitStack() as stk:
            consts = stk.enter_context(tc.tile_pool(name="consts", bufs=1))
            state = stk.enter_context(tc.tile_pool(name="state", bufs=1))
            xring = stk.enter_context(tc.tile_pool(name="xring", bufs=8))
            work = stk.enter_context(tc.tile_pool(name="work", bufs=3))
            pg = stk.enter_context(tc.tile_pool(name="pg", bufs=2, space="PSUM"))
            ptr = stk.enter_context(tc.tile_pool(name="ptr", bufs=1, space="PSUM"))
            pfc = stk.enter_context(tc.tile_pool(name="pfc", bufs=1, space="PSUM"))

            # ---- constants
            wt = consts.tile([128, KC, NH], mm_dt)
            nc.sync.dma_start(out=wt[:], in_=wt_d[:])
            wfc = consts.tile([128, KC_H, O], mm_dt)
            nc.sync.dma_start(out=wfc[:], in_=wfc_d[:])
            bias = consts.tile([G, NH // G], mm_dt)
            nc.sync.dma_start(out=bias[:], in_=bias_d[:])
            emat = consts.tile([G, 128], mm_dt)
            nc.sync.dma_start(out=emat[:], in_=e_d[:])
            cmr = consts.tile([RP, RF], f32)
            nc.sync.dma_start(out=cmr[:], in_=cm_d[:])
            hmr = consts.tile([RP, RF], f32)
            nc.sync.dma_start(out=hmr[:], in_=hm_d[:])
            hmT = consts.tile([128, 128], f32)
            nc.sync.dma_start(out=hmT[:], in_=hmT_d[:])
            om = consts.tile([BL, O], f32)
            nc.sync.dma_start(out=om[:], in_=om_d[:])
            ident = consts.tile([128, 128], f32)
            nc.sync.dma_start(out=ident[:], in_=id_d[:])


            # ---- persistent state
            hTm = state.tile([128, 128], mm_dt)      # masked h, transposed
            nc.sync.dma_start(out=hTm[:], in_=hT0_d[:])
            zT = state.tile([128, 64], mm_dt)        # masked z, transposed
            nc.sync.dma_start(out=zT[:], in_=zT0_d[:])
            c = state.tile([RP, RF], f32)            # cell state (masked)
            nc.sync.dma_start(out=c[:], in_=c0_d[:])
            h_raw = state.tile([RP, RF], f32)        # pre-mask h (last step)

            def step(iv):
                # -------- gate matmuls into ONE PSUM tile [128, 512]
                # partition 32n+b, free col 128g+f  (gate g, hidden 128n+f)
                g_ps = pg.tile([128, 512], f32, tag="gates", name="gates")

                xt = xring.tile([128, KC_X, BL], mm_dt, tag="xt")
                with tc.high_priority(offset=PRIO_HOIST):
                    nc.sync.dma_start(out=xt[:], in_=xT_d[bass.ts(iv, 1)])

                def stat(k):
                    if k in XK:
                        return xt[:, k, :]
                    if k in ZK:
                        kz = k - KC_X
                        return zT[:, 32 * kz:32 * kz + 32]
                    kh = k - KC_X - KC_Z
                    col = _hcol(kh, G)
                    return hTm[:, col:col + 32]

                # contraction chunks: x first, then h, then z (z arrives last)
                # skip_group_check: CoreSim's zero-region bookkeeping
                # mis-translates partition-offset (col-tiled) windows; the
                # data path is correct.
                order = XK + HK + ZK
                for idx, k in enumerate(order):
                    st = stat(k)
                    last = idx == len(order) - 1
                    import contextlib as _cl
                    # (x-matmul priority hoist measured as a wash: it breaks
                    # LDWEIGHTS/matmul round pipelining — keep program order)
                    with _cl.nullcontext():
                        for n in range(G):
                            nc.tensor.matmul(
                                out=g_ps[32 * n:32 * n + 32, :],
                                lhsT=mmc(st),
                                rhs=mmc(wt[:, k, 512 * n:512 * n + 512]),
                                start=(idx == 0), stop=last,
                                tile_position=(0, 32 * n),
                                # group bookkeeping windows are mis-translated
                                # for partition-offset (n>0) outputs; keep the
                                # check (and its mark/clear side effects) only
                                # on the n==0 start and stop.
                                skip_group_check=(
                                    n > 0 or (0 < idx < len(order) - 1)),
                            )
                # bias last: one full-tile K=G matmul
                # (emat[j, 32n+b] = 1 iff n == j  =>  rows 32n+b get bias[n])
                nc.tensor.matmul(out=g_ps[:], lhsT=mmc(emat[:]),
                                 rhs=mmc(bias[:]), start=False, stop=True,
                                 skip_group_check=True)

                # -------- activations: gates at free cols (i,f,o,g) order;
                # one sigmoid over [0:384], one tanh over [384:512]
                s_ifo = work.tile([RP, 384], f32, tag="sifo")
                nc.scalar.activation(out=s_ifo[:], in_=g_ps[:, 0:384],
                                     func=AF.Sigmoid, bias=0.0, scale=1.0)
                t_g = work.tile([RP, RF], f32, tag="tg")
                nc.scalar.activation(out=t_g[:], in_=g_ps[:, 384:512],
                                     func=AF.Tanh, bias=0.0, scale=1.0)
                s_i = s_ifo[:, 0:128]
                s_f = s_ifo[:, 128:256]
                s_o = s_ifo[:, 256:384]

                # -------- cell math (r-layout, fp32)
                tmp1 = work.tile([RP, RF], f32, tag="tmp1")
                nc.gpsimd.tensor_mul(out=tmp1[:], in0=s_f[:], in1=c[:])
                tmp2 = work.tile([RP, RF], f32, tag="tmp2")
                nc.vector.tensor_mul(out=tmp2[:], in0=s_i[:], in1=t_g[:])
                c_raw = work.tile([RP, RF], f32, tag="c_raw")
                nc.vector.tensor_add(out=c_raw[:], in0=tmp1[:], in1=tmp2[:])
                nc.gpsimd.tensor_mul(out=c[:], in0=c_raw[:], in1=cmr[:])
                t_c = work.tile([RP, RF], f32, tag="t_c")
                nc.scalar.activation(out=t_c[:], in_=c_raw[:],
                                     func=AF.Tanh, bias=0.0, scale=1.0)
                nc.vector.tensor_mul(out=h_raw[:], in0=s_o[:], in1=t_c[:])

                # -------- transpose h_raw -> hT psum [128, 128] (fp32)
                hT_ps = ptr.tile([128, 128], f32, tag="hT",
                                 padded_shape=[128, 512])
                if G == 4:
                    nc.tensor.matmul(out=hT_ps[:], lhsT=h_raw[:],
                                     rhs=ident[:], is_transpose=True)
                else:
                    nc.tensor.matmul(out=hT_ps[:, 0:64], lhsT=h_raw[:, 0:128],
                                     rhs=ident[0:64, 0:64], is_transpose=True,
                                     start=True, stop=False)
                    nc.tensor.matmul(out=hT_ps[:, 64:128],
                                     lhsT=h_raw[:, 128:256],
                                     rhs=ident[0:64, 0:64], is_transpose=True,
                                     start=False, stop=True)

                # evictions: unmasked (fc stationary) + masked (next gates)
                hTr = work.tile([128, 128], mm_dt, tag="hTr")
                nc.scalar.copy(out=hTr[:], in_=hT_ps[:])
                nc.vector.tensor_mul(out=hTm[:], in0=hT_ps[:], in1=hmT[:])

                # -------- fc: z_pre = h_raw @ W_fc.T   [BL, O]
                fc_ps = pfc.tile([BL, O], f32, tag="fc",
                                 padded_shape=[128, 512])
                for k in range(KC_H):
                    col = _hcol(k, G)
                    nc.tensor.matmul(
                        out=fc_ps[:],
                        lhsT=mmc(hTr[:, col:col + 32]),
                        rhs=mmc(wfc[:, k, :]),
                        start=(k == 0), stop=(k == KC_H - 1),
                        tile_position=(0, 0),
                    )
                t_fc = work.tile([BL, O], f32, tag="t_fc")
                nc.scalar.activation(out=t_fc[:], in_=fc_ps[:],
                                     func=AF.Tanh, bias=0.0, scale=1.0)
                z_m = work.tile([BL, O], f32, tag="z_m")
                nc.vector.tensor_mul(out=z_m[:], in0=t_fc[:], in1=om[:])
                nc.sync.dma_start(out=z_out[bass.ts(iv, 1)], in_=z_m[:])

                # -------- transpose z -> zT [128, 64]
                zT_ps = ptr.tile([128, 64], f32, tag="zT",
                                 padded_shape=[128, 512])
                nc.tensor.matmul(out=zT_ps[:, 0:32], lhsT=z_m[:, 0:128],
                                 rhs=ident[0:32, 0:32], is_transpose=True,
                                 start=True, stop=False)
                nc.tensor.matmul(out=zT_ps[:, 32:64], lhsT=z_m[:, 128:256],
                                 rhs=ident[0:32, 0:32], is_transpose=True,
                                 start=False, stop=True)
                nc.vector.tensor_copy(out=zT[:], in_=zT_ps[:])

            if unroll >= nsteps:
                for t_i in range(nsteps):
                    step(t_i)
            else:
                assert nsteps % unroll == 0
                tc.For_i_unrolled(0, nsteps, 1, step, max_unroll=unroll)

            # -------- final outputs
            h_m = state.tile([RP, RF], f32)
            nc.vector.tensor_mul(out=h_m[:], in0=h_raw[:], in1=hmr[:])
            nc.sync.dma_start(out=h_out[:], in_=h_m[:])
            nc.sync.dma_start(out=c_out[:], in_=c[:])

    nc.finalize()
    return nc


def _get_program(nsteps, mode, unroll):
    key = (nsteps, mode, unroll)
    if key not in _PROGRAM_CACHE:
        _PROGRAM_CACHE[key] = _build(nsteps, mode, unroll)
    return _PROGRAM_CACHE[key]


# ---------------------------------------------------------------- entry

def make_in_maps(inputs, nsteps, mode):
    """Full inputs -> per-core input dicts (host-side shard + relayout)."""
    cfg = _cfg(mode)
    G, mm_np = cfg["G"], cfg["mm_np"]

    x = np.asarray(inputs["inputs"], np.float32)[:nsteps]
    h0 = np.asarray(inputs["h0"], np.float32)
    c0 = np.asarray(inputs["c0"], np.float32)
    z0 = np.asarray(inputs["z0"], np.float32)
    om = np.asarray(inputs["out_mask"], np.float32)
    hm = np.asarray(inputs["h_mask"], np.float32)
    cm = np.asarray(inputs["c_mask"], np.float32)
    W_ih = np.asarray(inputs["W_ih"], np.float32)
    W_hh = np.asarray(inputs["W_hh"], np.float32)
    b = (np.asarray(inputs["b_ih"], np.float32)
         + np.asarray(inputs["b_hh"], np.float32))
    W_fc = np.asarray(inputs["W_fc"], np.float32)

    # combined weights, chunk order x, z, h
    W_comb = np.concatenate([W_ih[:, :I], W_ih[:, I:], W_hh], axis=1)  # [NH, 1024]
    # gate reorder (i,f,g,o)->(i,f,o,g) so one sigmoid spans 3 gates, then
    # (g, n) swap of the gate-column axis: new col 512n+128g'+f
    GP = [0, 1, 3, 2]
    W_comb = np.ascontiguousarray(
        W_comb.reshape(4, G, NH // (4 * G), I + O + H)[GP]
        .transpose(1, 0, 2, 3).reshape(NH, I + O + H))
    b_r = np.ascontiguousarray(
        b.reshape(4, G, NH // (4 * G))[GP].transpose(1, 0, 2).reshape(G, NH // G))
    wt = np.ascontiguousarray(
        W_comb.T.reshape(KC, 128, NH).transpose(1, 0, 2)).astype(mm_np)
    wfc = np.ascontiguousarray(
        W_fc.T.reshape(KC_H, 128, O).transpose(1, 0, 2)).astype(mm_np)
    bias = b_r.astype(mm_np)
    emat = np.zeros((G, 128), np.float32)
    for j in range(G):
        emat[j, 32 * j:32 * j + 32] = 1.0
    ident = np.eye(128, dtype=np.float32)

    in_maps = []
    for j in range(NCORES):
        sl = slice(BL * j, BL * (j + 1))
        xc = x[:, sl, :]                                     # [T, BL, I]
        xT = np.ascontiguousarray(
            xc.reshape(nsteps, BL, KC_X, 128).transpose(0, 3, 2, 1)
        ).astype(mm_np)                                      # [T,128,KC_X,BL]
        in_maps.append(dict(
            xT=xT, wt=wt, wfc=wfc, bias=bias, emat=emat.astype(mm_np),
            hT0=_hT_layout(h0[sl], G).astype(mm_np),
            zT0=_zT_layout(z0[sl]).astype(mm_np),
            c0r=_to_r(c0[sl], G),
            cmr=_to_r(cm[sl], G),
            hmr=_to_r(hm[sl], G),
            hmT=_hT_layout(hm[sl], G).astype(np.float32),
            om=np.ascontiguousarray(om[sl]),
            ident=ident,
        ))
    return in_maps


def assemble_outputs(results, nsteps, mode):
    G = _cfg(mode)["G"]
    outs = np.zeros((nsteps, B, O), np.float32)
    h_f = np.zeros((B, H), np.float32)
    c_f = np.zeros((B, H), np.float32)
    for j, r in enumerate(results):
        sl = slice(BL * j, BL * (j + 1))
        outs[:, sl, :] = r["z_out"]
        h_f[sl] = _from_r(r["h_out"], G)
        c_f[sl] = _from_r(r["c_out"], G)
    return outs, (h_f, c_f)


def run(inputs, nsteps=T, mode=MODE, unroll=UNROLL, **spmd_kwargs):
    from concourse.bass_utils import run_bass_kernel_spmd

    nc = _get_program(nsteps, mode, unroll)
    in_maps = make_in_maps(inputs, nsteps, mode)
    res = run_bass_kernel_spmd(nc, in_maps, core_ids=list(range(NCORES)),
                               **spmd_kwargs)
    return res, assemble_outputs(res.results, nsteps, mode)


def kernel(**inputs):
    _res, out = run(inputs)
    return out
